# revision 11
# baseline (speedup 1.0000x reference)
"""3D Haar DWT (single level) on Trainium2, data-parallel over 8 NeuronCores.

Input  x: (2, 32, 64, 128, 128) f32  -> 8 subbands, each (2, 32, 32, 64, 64).

Design (per core; 8 of the 64 (N*C) volumes each):
  The host splits x into fp16 hi + lo halves (x * 2^10 = hi + lo + O(2^-12)
  relative), so the PE can run 1-cycle/row fp16 matmuls while keeping
  fp32-level end-to-end precision.  The H-axis transform matrix is reduced
  to its exact {0,+-1} sign pattern (fp16-exact); all scale factors
  (1/sqrt2 per axis and the 2^-10 split scale) fold into one fp32 multiply
  at PSUM eviction.

  Per group of 4 consecutive D-slices (= 2 output d-pairs) of a volume:
    1. two 128 KiB DMAs bring the hi and lo fp16 slices into SBUF,
    2. eight fp16 PE matmuls compute the H transform AND the D-axis
       butterfly via PSUM accumulation: even slices add into both D bands,
       odd slices add into the low band and subtract (negated sign matrix)
       from the high band,
    3. ACT evicts PSUM -> SBUF applying the folded scale,
    4. DVE does the W-axis butterfly (even +- odd columns), writing the
       final output tile,
    5. one 256 KiB DMA writes the 2 finished d-pair outputs to DRAM.
"""

import os
import sys

import numpy as np

for _p in ("/opt/trn_rl_repo", "/root/.axon_site/_ro/trn_rl_repo"):
    if os.path.isdir(_p) and _p not in sys.path:
        sys.path.append(_p)

N, C, D, H, W = 2, 32, 64, 128, 128
G = N * C            # 64 independent (D, H, W) volumes
N_CORES = 8
GPC = G // N_CORES   # 8 volumes per core
R = D // 2
SLICES = 4           # D-slices per iteration (= 2 output d-pairs)
PAIRS = SLICES // 2
ITERS = D // SLICES
SPLIT_SCALE = 1024.0  # 2^10: keeps fp16 hi/lo away from subnormals

_CACHE = {}


def _build_program(scale: float):
    import concourse.bacc as bacc
    import concourse.mybir as mybir
    import concourse.tile as tile
    from contextlib import ExitStack

    f32 = mybir.dt.float32
    f16 = mybir.dt.float16
    add = mybir.AluOpType.add
    sub = mybir.AluOpType.subtract

    nc = bacc.Bacc(
        "TRN2",
        target_bir_lowering=False,
        debug=False,
        num_devices=N_CORES,
    )

    xhd = nc.dram_tensor("xh", [GPC, D, H, W], f16, kind="ExternalInput")
    xld = nc.dram_tensor("xl", [GPC, D, H, W], f16, kind="ExternalInput")
    mpd = nc.dram_tensor("mp", [H, 128], f16, kind="ExternalInput")
    mnd = nc.dram_tensor("mn", [H, 128], f16, kind="ExternalInput")
    # y[g, r, ps, db, qs]: ps = h-band*64 + p, db = d-band, qs = w-band*64 + q
    yd = nc.dram_tensor("y", [GPC, R, 128, 2, 128], f32, kind="ExternalOutput")

    with ExitStack() as ctx:
        tc = ctx.enter_context(tile.TileContext(nc))
        const = ctx.enter_context(tc.tile_pool(name="const", bufs=1))
        mpt = const.tile([H, 128], f16, tag="mp")
        mnt = const.tile([H, 128], f16, tag="mn")
        nc.sync.dma_start(mpt[:], mpd[:])
        nc.sync.dma_start(mnt[:], mnd[:])

        xp = ctx.enter_context(tc.tile_pool(name="xp", bufs=5))
        p1 = ctx.enter_context(tc.tile_pool(name="p1", bufs=5, space="PSUM"))
        s1 = ctx.enter_context(tc.tile_pool(name="s1", bufs=5))
        s2 = ctx.enter_context(tc.tile_pool(name="s2", bufs=5))

        for g in range(GPC):
            for it in range(ITERS):
                s0 = it * SLICES
                xh = xp.tile([H, SLICES, W], f16, tag="xh")
                xl = xp.tile([H, SLICES, W], f16, tag="xl")
                nc.sync.dma_start(
                    xh[:], xhd[g, s0 : s0 + SLICES].rearrange("s h w -> h s w")
                )
                nc.sync.dma_start(
                    xl[:], xld[g, s0 : s0 + SLICES].rearrange("s h w -> h s w")
                )
                # o1 free layout: (pr, db, w); even slices (s=0,2) are the
                # d-pairs' first members, odd (s=1,3) the second.
                o1 = p1.tile([128, PAIRS, 2, W], f32, tag="o1")
                lo = o1[:, :, 0, :]   # d-band low half
                hi = o1[:, :, 1, :]   # d-band high half
                # start=True zeroes the whole 2 KiB PSUM zero-region, so only
                # the very first matmul of this tile may carry it; all later
                # ones rely on per-element has_written (clear -> overwrite,
                # set -> accumulate).
                mms = [
                    (lo, mpt, 0), (hi, mpt, 0),   # even slices: + into both
                    (lo, mpt, 1), (hi, mnt, 1),   # odd: + into low, - into high
                ]
                for k, xt in enumerate((xh, xl)):
                    for j, (dst, m, par) in enumerate(mms):
                        nc.tensor.matmul(
                            dst,
                            m[:],
                            xt[:, par::2, :],
                            start=(k == 0 and j == 0),
                            stop=(k == 1 and j == len(mms) - 1),
                            skip_group_check=True,
                        )

                # evict PSUM -> SBUF with the folded scale
                o1s = s1.tile([128, PAIRS, 2, W], f32, tag="o1s")
                nc.scalar.mul(o1s[:], o1[:], float(scale))

                # W butterfly (stride-2 reads): out[pr, db, wb, j]
                o2s = s2.tile([128, PAIRS, 2, 2, 64], f32, tag="o2s")
                i0 = o1s[:].rearrange("p a b (j t) -> p a b j t", t=2)
                nc.vector.tensor_tensor(
                    o2s[:, :, :, 0, :], i0[:, :, :, :, 0], i0[:, :, :, :, 1], add
                )
                nc.vector.tensor_tensor(
                    o2s[:, :, :, 1, :], i0[:, :, :, :, 0], i0[:, :, :, :, 1], sub
                )

                r0 = s0 // 2
                nc.sync.dma_start(
                    yd[g, r0 : r0 + PAIRS].rearrange("r p a q -> p r a q"),
                    o2s[:],
                )

    nc.compile()
    return nc


def kernel(x, matrix_low_0, matrix_low_1, matrix_low_2,
           matrix_high_0, matrix_high_1, matrix_high_2):
    from concourse.bass_utils import run_bass_kernel_spmd

    x = np.asarray(x, dtype=np.float32)
    mh0 = np.asarray(matrix_low_0, dtype=np.float32)    # (64, 128)
    mg0 = np.asarray(matrix_high_0, dtype=np.float32)   # (64, 128)
    m1l = np.asarray(matrix_low_1, dtype=np.float32)    # (128, 64)
    ml2 = np.asarray(matrix_low_2, dtype=np.float32)    # (32, 64)

    # exact {0,+-1} sign pattern of the stacked H matrix (fp16-exact)
    sH = np.float64(np.abs(mh0[0, 0]))
    m0 = np.concatenate([mh0, mg0], axis=0).T           # (128 h, 128 p)
    mp = np.ascontiguousarray(np.sign(m0).astype(np.float16))
    mn = np.ascontiguousarray((-np.sign(m0)).astype(np.float16))

    # one folded output scale: sH * sW * sD / SPLIT_SCALE
    scale = float(
        np.float32(sH * np.float64(m1l[0, 0]) * np.float64(ml2[0, 0]) / SPLIT_SCALE)
    )

    # fp16 hi/lo split of x * 2^10 (exact power-of-two prescale)
    xs = x.reshape(G, D, H, W) * np.float32(SPLIT_SCALE)
    xh = xs.astype(np.float16)
    xl = (xs - xh.astype(np.float32)).astype(np.float16)

    key = ("prog", float(scale))
    if key not in _CACHE:
        _CACHE[key] = _build_program(float(scale))
    nc = _CACHE[key]

    in_maps = [
        {
            "xh": np.ascontiguousarray(xh[i * GPC : (i + 1) * GPC]),
            "xl": np.ascontiguousarray(xl[i * GPC : (i + 1) * GPC]),
            "mp": mp,
            "mn": mn,
        }
        for i in range(N_CORES)
    ]
    res = run_bass_kernel_spmd(nc, in_maps, list(range(N_CORES)))
    _CACHE["last_result"] = res
    y = np.concatenate([res.results[i]["y"] for i in range(N_CORES)], axis=0)
    # y: (64, 32, 128, 2, 128) = [g, r, (hb p), db, (wb q)]
    full = y.reshape(N, C, R, 2, H // 2, 2, 2, W // 2)  # n c r hb p db wb q
    out = np.transpose(full, (5, 3, 6, 0, 1, 2, 4, 7))  # db hb wb n c r p q
    out = np.ascontiguousarray(out).reshape(8, N, C, R, H // 2, W // 2)
    return tuple(out[s] for s in range(8))


# revision 15
# speedup vs baseline: 1.6601x; 1.6601x over previous
"""3D Haar DWT (single level) on Trainium2, data-parallel over 8 NeuronCores.

Input  x: (2, 32, 64, 128, 128) f32  -> 8 subbands, each (2, 32, 32, 64, 64).

Design (per core; 8 of the 64 (N*C) volumes each):
  The host splits x into fp16 hi + lo halves (x * 2^10 = hi + lo with
  ~2^-22 relative residual), row-interleaved in one tensor so input DMA
  runs stay 512 B.  The PE runs two 1-cycle/row fp16 matmuls per tile
  (hi and lo, accumulated in fp32 PSUM) against the exact {0,+-1} sign
  pattern of the stacked H-transform matrix; all scale factors fold into
  one fp32 multiply at PSUM eviction (ACT).  The W- and D-axis Haar
  butterflies are plain fp32 adds/subs on DVE (optionally one on GpSimd).

  Per group of 4 consecutive D-slices (= 2 output d-pairs) of a volume:
    1. one 256 KiB DMA brings hi+lo fp16 slices into SBUF,
    2. two fp16 PE matmuls (H transform, hi+lo accumulate),
    3. ACT evicts PSUM -> SBUF applying the folded scale,
    4. DVE does the W butterfly then the D butterfly,
    5. one 256 KiB DMA writes the 2 finished d-pair outputs to DRAM.
"""

import os
import sys

import numpy as np

for _p in ("/opt/trn_rl_repo", "/root/.axon_site/_ro/trn_rl_repo"):
    if os.path.isdir(_p) and _p not in sys.path:
        sys.path.append(_p)

N, C, D, H, W = 2, 32, 64, 128, 128
G = N * C            # 64 independent (D, H, W) volumes
N_CORES = 8
GPC = G // N_CORES   # 8 volumes per core
R = D // 2
SLICES = 4           # D-slices per iteration (= 2 output d-pairs)
PAIRS = SLICES // 2
ITERS = D // SLICES
SPLIT_SCALE = 1024.0  # 2^10: keeps fp16 hi/lo away from subnormals
D_ON_GPS = os.environ.get("DWT_D_ON_GPS", "0") == "1"

_CACHE = {}


def _build_program(scale: float):
    import concourse.bacc as bacc
    import concourse.mybir as mybir
    import concourse.tile as tile
    from contextlib import ExitStack

    f32 = mybir.dt.float32
    f16 = mybir.dt.float16
    add = mybir.AluOpType.add
    sub = mybir.AluOpType.subtract

    nc = bacc.Bacc(
        "TRN2",
        target_bir_lowering=False,
        debug=False,
        num_devices=N_CORES,
    )

    # hi/lo interleaved per (h) row: [..., h, 0, :] = hi, [..., h, 1, :] = lo
    xd = nc.dram_tensor("xhl", [GPC, D, H, 2, W], f16, kind="ExternalInput")
    mpd = nc.dram_tensor("mp", [H, 128], f16, kind="ExternalInput")
    # y[g, r, ps, db, qs]: ps = h-band*64 + p, db = d-band, qs = w-band*64 + q
    yd = nc.dram_tensor("y", [GPC, R, 128, 2, 128], f32, kind="ExternalOutput")

    with ExitStack() as ctx:
        tc = ctx.enter_context(tile.TileContext(nc))
        const = ctx.enter_context(tc.tile_pool(name="const", bufs=1))
        mpt = const.tile([H, 128], f16, tag="mp")
        nc.sync.dma_start(mpt[:], mpd[:])

        xp = ctx.enter_context(tc.tile_pool(name="xp", bufs=6))
        p1 = ctx.enter_context(tc.tile_pool(name="p1", bufs=6, space="PSUM"))
        s1 = ctx.enter_context(tc.tile_pool(name="s1", bufs=6))
        scr = ctx.enter_context(tc.tile_pool(name="scr", bufs=3))
        s2 = ctx.enter_context(tc.tile_pool(name="s2", bufs=6))

        for g in range(GPC):
            for it in range(ITERS):
                s0 = it * SLICES
                xt = xp.tile([H, SLICES, 2, W], f16, tag="xt")
                nc.sync.dma_start(
                    xt[:], xd[g, s0 : s0 + SLICES].rearrange("s h t w -> h s t w")
                )
                # o1 free layout: (s, w). Two big accumulating matmuls: hi + lo.
                o1 = p1.tile([128, SLICES, W], f32, tag="o1")
                nc.tensor.matmul(o1[:], mpt[:], xt[:, :, 0, :], start=True, stop=False)
                nc.tensor.matmul(o1[:], mpt[:], xt[:, :, 1, :], start=False, stop=True)

                # evict PSUM -> SBUF with the folded scale
                o1s = s1.tile([128, SLICES, W], f32, tag="o1s")
                nc.scalar.mul(o1s[:], o1[:], float(scale))

                # W butterfly on DVE (stride-2 reads): sc[s, wb, j]
                sc = scr.tile([128, SLICES, 2, 64], f32, tag="scr")
                i0 = o1s[:].rearrange("p s (j t) -> p s j t", t=2)
                nc.vector.tensor_tensor(
                    sc[:, :, 0, :], i0[:, :, :, 0], i0[:, :, :, 1], add
                )
                nc.vector.tensor_tensor(
                    sc[:, :, 1, :], i0[:, :, :, 0], i0[:, :, :, 1], sub
                )

                # D butterfly: o2s[pr, db, qs] = sc[2pr] +- sc[2pr+1]
                scv = sc[:].rearrange("p s wb j -> p s (wb j)")   # (128, 4, 128)
                o2s = s2.tile([128, PAIRS, 2, 128], f32, tag="o2s")
                nc.vector.tensor_tensor(
                    o2s[:, :, 0, :], scv[:, 0::2, :], scv[:, 1::2, :], add
                )
                eng = nc.gpsimd if D_ON_GPS else nc.vector
                eng.tensor_tensor(
                    o2s[:, :, 1, :], scv[:, 0::2, :], scv[:, 1::2, :], sub
                )

                # output DMAs ride the ACT HWDGE ring (qActDynamicHW) so a
                # stalled store never head-of-line-blocks later input DMAs
                # on the SP ring
                r0 = s0 // 2
                nc.scalar.dma_start(
                    yd[g, r0 : r0 + PAIRS].rearrange("r p a q -> p r a q"),
                    o2s[:],
                )

    nc.compile()
    return nc


def kernel(x, matrix_low_0, matrix_low_1, matrix_low_2,
           matrix_high_0, matrix_high_1, matrix_high_2):
    from concourse.bass_utils import run_bass_kernel_spmd

    x = np.asarray(x, dtype=np.float32)
    mh0 = np.asarray(matrix_low_0, dtype=np.float32)    # (64, 128)
    mg0 = np.asarray(matrix_high_0, dtype=np.float32)   # (64, 128)
    m1l = np.asarray(matrix_low_1, dtype=np.float32)    # (128, 64)
    ml2 = np.asarray(matrix_low_2, dtype=np.float32)    # (32, 64)

    # exact {0,+-1} sign pattern of the stacked H matrix (fp16-exact)
    sH = np.float64(np.abs(mh0[0, 0]))
    m0 = np.concatenate([mh0, mg0], axis=0).T           # (128 h, 128 p)
    mp = np.ascontiguousarray(np.sign(m0).astype(np.float16))

    # one folded output scale: sH * sW * sD / SPLIT_SCALE
    scale = float(
        np.float32(sH * np.float64(m1l[0, 0]) * np.float64(ml2[0, 0]) / SPLIT_SCALE)
    )

    # fp16 hi/lo split of x * 2^10 (exact power-of-two prescale),
    # row-interleaved so DMA descriptors stay 512 B
    xs = x.reshape(G, D, H, W) * np.float32(SPLIT_SCALE)
    xh = xs.astype(np.float16)
    xl = (xs - xh.astype(np.float32)).astype(np.float16)
    xhl = np.stack([xh, xl], axis=3)                    # (G, D, H, 2, W)

    key = ("prog", float(scale), D_ON_GPS)
    if key not in _CACHE:
        _CACHE[key] = _build_program(float(scale))
    nc = _CACHE[key]

    in_maps = [
        {
            "xhl": np.ascontiguousarray(xhl[i * GPC : (i + 1) * GPC]),
            "mp": mp,
        }
        for i in range(N_CORES)
    ]
    res = run_bass_kernel_spmd(nc, in_maps, list(range(N_CORES)))
    _CACHE["last_result"] = res
    y = np.concatenate([res.results[i]["y"] for i in range(N_CORES)], axis=0)
    # y: (64, 32, 128, 2, 128) = [g, r, (hb p), db, (wb q)]
    full = y.reshape(N, C, R, 2, H // 2, 2, 2, W // 2)  # n c r hb p db wb q
    out = np.transpose(full, (5, 3, 6, 0, 1, 2, 4, 7))  # db hb wb n c r p q
    out = np.ascontiguousarray(out).reshape(8, N, C, R, H // 2, W // 2)
    return tuple(out[s] for s in range(8))


# revision 16
# speedup vs baseline: 1.7032x; 1.0259x over previous
"""3D Haar DWT (single level) on Trainium2, data-parallel over 8 NeuronCores.

Input  x: (2, 32, 64, 128, 128) f32  -> 8 subbands, each (2, 32, 32, 64, 64).

Design (per core; 8 of the 64 (N*C) volumes each):
  The host splits x into fp16 hi + lo halves (x * 2^10 = hi + lo with
  ~2^-22 relative residual), row-interleaved in one tensor so input DMA
  runs stay 512 B.  The PE runs 1-cycle/row fp16 matmuls (hi and lo,
  accumulated in fp32 PSUM) against the exact {0,+-1} sign pattern of the
  stacked H-transform matrix; the matmul's moving-operand access pattern
  also de-interleaves even/odd W columns (free on the PE -- column order
  only).  All scale factors fold into one fp32 multiply at PSUM eviction
  (ACT).  The W- and D-axis Haar butterflies are contiguous fp32
  adds/subs on DVE.  Output DMAs ride the ACT HWDGE ring so stores never
  head-of-line-block loads on the SP ring.

  Per group of 8 consecutive D-slices (= 4 output d-pairs) of a volume:
    1. one 512 KiB DMA brings hi+lo fp16 slices into SBUF,
    2. 4 fp16 PE matmuls (H transform, hi+lo accumulate, 2 PSUM banks),
    3. 2 ACT evictions PSUM -> SBUF applying the folded scale,
    4. 2+2 DVE butterflies (W then D axis),
    5. one 512 KiB DMA writes the 4 finished d-pair outputs to DRAM.
"""

import os
import sys

import numpy as np

for _p in ("/opt/trn_rl_repo", "/root/.axon_site/_ro/trn_rl_repo"):
    if os.path.isdir(_p) and _p not in sys.path:
        sys.path.append(_p)

N, C, D, H, W = 2, 32, 64, 128, 128
G = N * C            # 64 independent (D, H, W) volumes
N_CORES = 8
GPC = G // N_CORES   # 8 volumes per core
R = D // 2
SLICES = 8           # D-slices per iteration (= 4 output d-pairs)
HALF = SLICES // 2
PAIRS = SLICES // 2
ITERS = D // SLICES
SPLIT_SCALE = 1024.0  # 2^10: keeps fp16 hi/lo away from subnormals

_CACHE = {}


def _build_program(scale: float):
    import concourse.bacc as bacc
    import concourse.mybir as mybir
    import concourse.tile as tile
    from contextlib import ExitStack

    f32 = mybir.dt.float32
    f16 = mybir.dt.float16
    add = mybir.AluOpType.add
    sub = mybir.AluOpType.subtract

    nc = bacc.Bacc(
        "TRN2",
        target_bir_lowering=False,
        debug=False,
        num_devices=N_CORES,
    )

    # hi/lo interleaved per (h) row: [..., h, 0, :] = hi, [..., h, 1, :] = lo
    xd = nc.dram_tensor("xhl", [GPC, D, H, 2, W], f16, kind="ExternalInput")
    mpd = nc.dram_tensor("mp", [H, 128], f16, kind="ExternalInput")
    # y[g, r, ps, db, qs]: ps = h-band*64 + p, db = d-band, qs = w-band*64 + q
    yd = nc.dram_tensor("y", [GPC, R, 128, 2, 128], f32, kind="ExternalOutput")

    with ExitStack() as ctx:
        tc = ctx.enter_context(tile.TileContext(nc))
        const = ctx.enter_context(tc.tile_pool(name="const", bufs=1))
        mpt = const.tile([H, 128], f16, tag="mp")
        nc.sync.dma_start(mpt[:], mpd[:])

        xp = ctx.enter_context(tc.tile_pool(name="xp", bufs=4))
        p1 = ctx.enter_context(tc.tile_pool(name="p1", bufs=6, space="PSUM"))
        s1 = ctx.enter_context(tc.tile_pool(name="s1", bufs=4))
        scr = ctx.enter_context(tc.tile_pool(name="scr", bufs=2))
        s2 = ctx.enter_context(tc.tile_pool(name="s2", bufs=4))

        for g in range(GPC):
            for it in range(ITERS):
                s0 = it * SLICES
                xt = xp.tile([H, SLICES, 2, W], f16, tag="xt")
                nc.sync.dma_start(
                    xt[:], xd[g, s0 : s0 + SLICES].rearrange("s h t w -> h s t w")
                )
                # one PSUM bank per 4-slice half; moving-operand AP orders
                # columns as (s, eo, j) so o1 comes out de-interleaved
                o1s_t = s1.tile([128, SLICES, 2, 64], f32, tag="o1s")
                for hb in range(2):
                    sl = slice(hb * HALF, (hb + 1) * HALF)
                    o1 = p1.tile([128, HALF, 2, 64], f32, tag="o1")
                    rhs_hi = xt[:, sl, 0, :].rearrange("h s (j t) -> h s t j", t=2)
                    rhs_lo = xt[:, sl, 1, :].rearrange("h s (j t) -> h s t j", t=2)
                    nc.tensor.matmul(o1[:], mpt[:], rhs_hi, start=True, stop=False)
                    nc.tensor.matmul(o1[:], mpt[:], rhs_lo, start=False, stop=True)
                    # evict PSUM -> SBUF with the folded scale
                    nc.scalar.mul(o1s_t[:, sl, :, :], o1[:], float(scale))

                # W butterfly on DVE (contiguous): sc[s, wb, j]
                sc = scr.tile([128, SLICES, 2, 64], f32, tag="scr")
                nc.vector.tensor_tensor(
                    sc[:, :, 0, :], o1s_t[:, :, 0, :], o1s_t[:, :, 1, :], add
                )
                nc.vector.tensor_tensor(
                    sc[:, :, 1, :], o1s_t[:, :, 0, :], o1s_t[:, :, 1, :], sub
                )

                # D butterfly: o2s[pr, db, qs] = sc[2pr] +- sc[2pr+1]
                scv = sc[:].rearrange("p s wb j -> p s (wb j)")   # (128, 8, 128)
                o2s = s2.tile([128, PAIRS, 2, 128], f32, tag="o2s")
                nc.vector.tensor_tensor(
                    o2s[:, :, 0, :], scv[:, 0::2, :], scv[:, 1::2, :], add
                )
                nc.vector.tensor_tensor(
                    o2s[:, :, 1, :], scv[:, 0::2, :], scv[:, 1::2, :], sub
                )

                # output DMAs ride the ACT HWDGE ring (qActDynamicHW) so a
                # stalled store never head-of-line-blocks later input DMAs
                # on the SP ring
                r0 = s0 // 2
                nc.scalar.dma_start(
                    yd[g, r0 : r0 + PAIRS].rearrange("r p a q -> p r a q"),
                    o2s[:],
                )

    nc.compile()
    return nc


def kernel(x, matrix_low_0, matrix_low_1, matrix_low_2,
           matrix_high_0, matrix_high_1, matrix_high_2):
    from concourse.bass_utils import run_bass_kernel_spmd

    x = np.asarray(x, dtype=np.float32)
    mh0 = np.asarray(matrix_low_0, dtype=np.float32)    # (64, 128)
    mg0 = np.asarray(matrix_high_0, dtype=np.float32)   # (64, 128)
    m1l = np.asarray(matrix_low_1, dtype=np.float32)    # (128, 64)
    ml2 = np.asarray(matrix_low_2, dtype=np.float32)    # (32, 64)

    # exact {0,+-1} sign pattern of the stacked H matrix (fp16-exact)
    sH = np.float64(np.abs(mh0[0, 0]))
    m0 = np.concatenate([mh0, mg0], axis=0).T           # (128 h, 128 p)
    mp = np.ascontiguousarray(np.sign(m0).astype(np.float16))

    # one folded output scale: sH * sW * sD / SPLIT_SCALE
    scale = float(
        np.float32(sH * np.float64(m1l[0, 0]) * np.float64(ml2[0, 0]) / SPLIT_SCALE)
    )

    # fp16 hi/lo split of x * 2^10 (exact power-of-two prescale),
    # row-interleaved so DMA descriptors stay 512 B
    xs = x.reshape(G, D, H, W) * np.float32(SPLIT_SCALE)
    xh = xs.astype(np.float16)
    xl = (xs - xh.astype(np.float32)).astype(np.float16)
    xhl = np.stack([xh, xl], axis=3)                    # (G, D, H, 2, W)

    key = ("prog", float(scale))
    if key not in _CACHE:
        _CACHE[key] = _build_program(float(scale))
    nc = _CACHE[key]

    in_maps = [
        {
            "xhl": np.ascontiguousarray(xhl[i * GPC : (i + 1) * GPC]),
            "mp": mp,
        }
        for i in range(N_CORES)
    ]
    res = run_bass_kernel_spmd(nc, in_maps, list(range(N_CORES)))
    _CACHE["last_result"] = res
    y = np.concatenate([res.results[i]["y"] for i in range(N_CORES)], axis=0)
    # y: (64, 32, 128, 2, 128) = [g, r, (hb p), db, (wb q)]
    full = y.reshape(N, C, R, 2, H // 2, 2, 2, W // 2)  # n c r hb p db wb q
    out = np.transpose(full, (5, 3, 6, 0, 1, 2, 4, 7))  # db hb wb n c r p q
    out = np.ascontiguousarray(out).reshape(8, N, C, R, H // 2, W // 2)
    return tuple(out[s] for s in range(8))


# revision 17
# speedup vs baseline: 1.7656x; 1.0367x over previous
"""3D Haar DWT (single level) on Trainium2, data-parallel over 8 NeuronCores.

Input  x: (2, 32, 64, 128, 128) f32  -> 8 subbands, each (2, 32, 32, 64, 64).

Design (per core; 8 of the 64 (N*C) volumes each):
  The host splits x into fp16 hi + lo halves (x * 2^10 = hi + lo with
  ~2^-22 relative residual), row-interleaved in one tensor so input DMA
  runs stay 512 B.  The PE runs 1-cycle/row fp16 matmuls (hi and lo,
  accumulated in fp32 PSUM) against the exact {0,+-1} sign pattern of the
  stacked H-transform matrix; the matmul's moving-operand access pattern
  also de-interleaves even/odd W columns (free on the PE -- column order
  only).  All scale factors fold into one fp32 multiply at PSUM eviction
  (ACT).  The W- and D-axis Haar butterflies are contiguous fp32
  adds/subs on DVE.  Output DMAs ride the ACT HWDGE ring so stores never
  head-of-line-block loads on the SP ring.

  Per group of 8 consecutive D-slices (= 4 output d-pairs) of a volume:
    1. one 512 KiB DMA brings hi+lo fp16 slices into SBUF,
    2. 4 fp16 PE matmuls (H transform, hi+lo accumulate, 2 PSUM banks),
    3. 2 ACT evictions PSUM -> SBUF applying the folded scale,
    4. 2+2 DVE butterflies (W then D axis),
    5. one 512 KiB DMA writes the 4 finished d-pair outputs to DRAM.
"""

import os
import sys

import numpy as np

for _p in ("/opt/trn_rl_repo", "/root/.axon_site/_ro/trn_rl_repo"):
    if os.path.isdir(_p) and _p not in sys.path:
        sys.path.append(_p)

N, C, D, H, W = 2, 32, 64, 128, 128
G = N * C            # 64 independent (D, H, W) volumes
N_CORES = 8
GPC = G // N_CORES   # 8 volumes per core
R = D // 2
SLICES = 8           # D-slices per iteration (= 4 output d-pairs)
HALF = SLICES // 2
PAIRS = SLICES // 2
ITERS = D // SLICES
SPLIT_SCALE = 1024.0  # 2^10: keeps fp16 hi/lo away from subnormals

_CACHE = {}


def _build_program(scale: float):
    import concourse.bacc as bacc
    import concourse.mybir as mybir
    import concourse.tile as tile
    from contextlib import ExitStack

    f32 = mybir.dt.float32
    f16 = mybir.dt.float16
    add = mybir.AluOpType.add
    sub = mybir.AluOpType.subtract

    nc = bacc.Bacc(
        "TRN2",
        target_bir_lowering=False,
        debug=False,
        num_devices=N_CORES,
    )

    # hi/lo interleaved per (h) row: [..., h, 0, :] = hi, [..., h, 1, :] = lo
    xd = nc.dram_tensor("xhl", [GPC, D, H, 2, W], f16, kind="ExternalInput")
    mpd = nc.dram_tensor("mp", [H, 128], f16, kind="ExternalInput")
    # y[g, r, ps, db, qs]: ps = h-band*64 + p, db = d-band, qs = w-band*64 + q
    yd = nc.dram_tensor("y", [GPC, R, 128, 2, 128], f32, kind="ExternalOutput")

    with ExitStack() as ctx:
        tc = ctx.enter_context(tile.TileContext(nc))
        const = ctx.enter_context(tc.tile_pool(name="const", bufs=1))
        mpt = const.tile([H, 128], f16, tag="mp")
        nc.sync.dma_start(mpt[:], mpd[:])

        xp = ctx.enter_context(tc.tile_pool(name="xp", bufs=6))
        p1 = ctx.enter_context(tc.tile_pool(name="p1", bufs=7, space="PSUM"))
        s1 = ctx.enter_context(tc.tile_pool(name="s1", bufs=6))
        scr = ctx.enter_context(tc.tile_pool(name="scr", bufs=3))
        s2 = ctx.enter_context(tc.tile_pool(name="s2", bufs=6))

        for g in range(GPC):
            for it in range(ITERS):
                s0 = it * SLICES
                xt = xp.tile([H, SLICES, 2, W], f16, tag="xt")
                nc.sync.dma_start(
                    xt[:], xd[g, s0 : s0 + SLICES].rearrange("s h t w -> h s t w")
                )
                # one PSUM bank per 4-slice half; moving-operand AP orders
                # columns as (s, eo, j) so o1 comes out de-interleaved
                o1s_t = s1.tile([128, SLICES, 2, 64], f32, tag="o1s")
                for hb in range(2):
                    sl = slice(hb * HALF, (hb + 1) * HALF)
                    o1 = p1.tile([128, HALF, 2, 64], f32, tag="o1")
                    rhs_hi = xt[:, sl, 0, :].rearrange("h s (j t) -> h s t j", t=2)
                    rhs_lo = xt[:, sl, 1, :].rearrange("h s (j t) -> h s t j", t=2)
                    nc.tensor.matmul(o1[:], mpt[:], rhs_hi, start=True, stop=False)
                    nc.tensor.matmul(o1[:], mpt[:], rhs_lo, start=False, stop=True)
                    # evict PSUM -> SBUF with the folded scale
                    nc.scalar.mul(o1s_t[:, sl, :, :], o1[:], float(scale))

                # W butterfly on DVE (contiguous): sc[s, wb, j]
                sc = scr.tile([128, SLICES, 2, 64], f32, tag="scr")
                nc.vector.tensor_tensor(
                    sc[:, :, 0, :], o1s_t[:, :, 0, :], o1s_t[:, :, 1, :], add
                )
                nc.vector.tensor_tensor(
                    sc[:, :, 1, :], o1s_t[:, :, 0, :], o1s_t[:, :, 1, :], sub
                )

                # D butterfly: o2s[pr, db, qs] = sc[2pr] +- sc[2pr+1]
                scv = sc[:].rearrange("p s wb j -> p s (wb j)")   # (128, 8, 128)
                o2s = s2.tile([128, PAIRS, 2, 128], f32, tag="o2s")
                nc.vector.tensor_tensor(
                    o2s[:, :, 0, :], scv[:, 0::2, :], scv[:, 1::2, :], add
                )
                nc.vector.tensor_tensor(
                    o2s[:, :, 1, :], scv[:, 0::2, :], scv[:, 1::2, :], sub
                )

                # output DMAs ride the ACT HWDGE ring (qActDynamicHW) so a
                # stalled store never head-of-line-blocks later input DMAs
                # on the SP ring
                r0 = s0 // 2
                nc.scalar.dma_start(
                    yd[g, r0 : r0 + PAIRS].rearrange("r p a q -> p r a q"),
                    o2s[:],
                )

    nc.compile()
    return nc


def kernel(x, matrix_low_0, matrix_low_1, matrix_low_2,
           matrix_high_0, matrix_high_1, matrix_high_2):
    from concourse.bass_utils import run_bass_kernel_spmd

    x = np.asarray(x, dtype=np.float32)
    mh0 = np.asarray(matrix_low_0, dtype=np.float32)    # (64, 128)
    mg0 = np.asarray(matrix_high_0, dtype=np.float32)   # (64, 128)
    m1l = np.asarray(matrix_low_1, dtype=np.float32)    # (128, 64)
    ml2 = np.asarray(matrix_low_2, dtype=np.float32)    # (32, 64)

    # exact {0,+-1} sign pattern of the stacked H matrix (fp16-exact)
    sH = np.float64(np.abs(mh0[0, 0]))
    m0 = np.concatenate([mh0, mg0], axis=0).T           # (128 h, 128 p)
    mp = np.ascontiguousarray(np.sign(m0).astype(np.float16))

    # one folded output scale: sH * sW * sD / SPLIT_SCALE
    scale = float(
        np.float32(sH * np.float64(m1l[0, 0]) * np.float64(ml2[0, 0]) / SPLIT_SCALE)
    )

    # fp16 hi/lo split of x * 2^10 (exact power-of-two prescale),
    # row-interleaved so DMA descriptors stay 512 B
    xs = x.reshape(G, D, H, W) * np.float32(SPLIT_SCALE)
    xh = xs.astype(np.float16)
    xl = (xs - xh.astype(np.float32)).astype(np.float16)
    xhl = np.stack([xh, xl], axis=3)                    # (G, D, H, 2, W)

    key = ("prog", float(scale))
    if key not in _CACHE:
        _CACHE[key] = _build_program(float(scale))
    nc = _CACHE[key]

    in_maps = [
        {
            "xhl": np.ascontiguousarray(xhl[i * GPC : (i + 1) * GPC]),
            "mp": mp,
        }
        for i in range(N_CORES)
    ]
    res = run_bass_kernel_spmd(nc, in_maps, list(range(N_CORES)))
    _CACHE["last_result"] = res
    y = np.concatenate([res.results[i]["y"] for i in range(N_CORES)], axis=0)
    # y: (64, 32, 128, 2, 128) = [g, r, (hb p), db, (wb q)]
    full = y.reshape(N, C, R, 2, H // 2, 2, 2, W // 2)  # n c r hb p db wb q
    out = np.transpose(full, (5, 3, 6, 0, 1, 2, 4, 7))  # db hb wb n c r p q
    out = np.ascontiguousarray(out).reshape(8, N, C, R, H // 2, W // 2)
    return tuple(out[s] for s in range(8))
